# revision 1
# baseline (speedup 1.0000x reference)
"""CRF negative-log-likelihood kernel for Trainium2 (Bass/Tile), 8-core SPMD.

Problem: emission [128, 512, 32] f32, length [128], target [128, 512],
transition [32, 32], start/end_transition [32] -> scalar f32
  sum_b (log_partition_b - log_score_b)

Strategy (data-parallel over batch, 16 sequences per core):
  * log_partition via the forward algorithm run in EXP space so each step is
    one real matmul on TensorE:  A_{t} = (W^T A_{t-1}) .* E_t
    with per-(t,b) pre-normalization E_t = exp(em_t) / sum_j exp(em_t)
    (the log of the dropped scale, c_{t,b} = log sum_j exp(em[t,b,:]),
    is accumulated separately and added back at the end).
  * Variable lengths use an absorbing extra tag "omega" (index 32):
    W[i, omega] = exp(end_i), W[omega, omega] = 1, and the per-step
    multiplier for omega is 1 on padded steps / 0 on real steps.  All mass
    transitions into omega exactly at t = length_b, carrying the
    end_transition weight; full-length sequences never enter omega and get
    end_transition applied in the final reduction instead.
  * Scan state is [33, 16]: real tags at partition offset 0, omega at
    offset 32 (both 32-aligned).  The per-step E tiles come from PE
    transposes of the normalized exp-emission slab (4 timesteps per
    [16,128] chunk); omega multipliers live in a [1, 512*16] row.
  * log_score needs only its SUM over the batch, so it is computed with
    one-hot / count-matrix contractions (no gathers): emission term via a
    masked one-hot multiply in a [128, 64] full-partition relayout of
    (b, t); transition term via C[i,j] = #(valid t: tgt_t=i, tgt_{t+1}=j)
    built from 64 PSUM-accumulated matmuls, dotted with raw T; start/end
    terms via tiny one-hot count matmuls.
  * Each core writes one partial sum; the host adds the 8 partials.
"""

import numpy as np

B = 16           # batch per core
S = 512          # sequence length
J = 32           # tags
JA = J + 1       # augmented with omega
NCORES = 8
CHUNK_T = 4      # timesteps per PE transpose chunk (4*32 = 128)
NCHUNK = S // CHUNK_T
P = 128          # full partition count for the score relayout
FS = B * S // P  # 64 free elems per partition in the score relayout


def build_bass(scan_steps=S, with_score=True):
    import concourse.bacc as bacc
    import concourse.tile as tile
    from concourse import mybir

    f32 = mybir.dt.float32
    i32 = mybir.dt.int32

    nc = bacc.Bacc(
        "TRN2", target_bir_lowering=False, debug=False, num_devices=NCORES
    )

    em_d = nc.dram_tensor("emission", [B, S, J], f32, kind="ExternalInput")
    len_d = nc.dram_tensor("length", [B, 1], i32, kind="ExternalInput")
    tgt_d = nc.dram_tensor("target", [B, S], i32, kind="ExternalInput")
    T_d = nc.dram_tensor("transition", [J, J], f32, kind="ExternalInput")
    st_d = nc.dram_tensor("start_transition", [J, 1], f32, kind="ExternalInput")
    en_d = nc.dram_tensor("end_transition", [J, 1], f32, kind="ExternalInput")
    om_d = nc.dram_tensor("omega", [1, S * B], f32, kind="ExternalInput")
    out_d = nc.dram_tensor("out", [1, 1], f32, kind="ExternalOutput")

    Exp = mybir.ActivationFunctionType.Exp
    Ln = mybir.ActivationFunctionType.Ln
    Alu = mybir.AluOpType
    Ax = mybir.AxisListType

    with tile.TileContext(nc) as tc:
        with (
            tc.tile_pool(name="big", bufs=1) as big,        # persistent slabs
            tc.tile_pool(name="small", bufs=1) as small,    # persistent small
            tc.tile_pool(name="apool", bufs=3) as apool,    # scan state
            tc.tile_pool(name="pscan", bufs=2, space="PSUM") as pscan,
            tc.tile_pool(name="ptrans", bufs=2, space="PSUM") as ptrans,
            tc.tile_pool(name="pfin", bufs=3, space="PSUM") as pfin,
        ):
            # ---------------- load inputs ----------------
            len_i = small.tile([B, 1], i32, tag="len_i")
            nc.sync.dma_start(len_i[:], len_d.ap())

            # ---------------- masks (b-partition layout) ----------------
            tvec = small.tile([B, S], i32, tag="tvec")
            nc.gpsimd.iota(tvec[:], pattern=[[1, S]], base=0,
                           channel_multiplier=0)
            len_f = small.tile([B, 1], f32, tag="len_f")
            nc.vector.tensor_copy(len_f[:], len_i[:])
            tvec_f = small.tile([B, S], f32, tag="tvec_f")
            nc.vector.tensor_copy(tvec_f[:], tvec[:])
            mask = small.tile([B, S], f32, tag="mask")
            nc.vector.tensor_scalar(
                mask[:], tvec_f[:], len_f[:], None, op0=Alu.is_lt
            )

            # ---------------- prep slab preT[b, t, j] ----------------
            preT = big.tile([B, S * J], f32, tag="preT")
            preT3 = preT[:].rearrange("b (s j) -> b s j", j=J)
            s_sum = small.tile([B, S], f32, tag="s_sum")
            TCK = 128
            for ck in range(S // TCK):
                sl = slice(ck * TCK, (ck + 1) * TCK)
                nc.sync.dma_start(preT3[:, sl, :], em_d.ap()[:, sl, :])
                nc.scalar.activation(preT3[:, sl, :], preT3[:, sl, :], Exp)
                nc.vector.tensor_reduce(
                    s_sum[:, sl], preT3[:, sl, :], axis=Ax.X, op=Alu.add
                )
            # s_eff = (s_sum - 1) * mask + 1   (=1 on padded steps)
            s_eff = small.tile([B, S], f32, tag="s_eff")
            nc.vector.tensor_scalar(s_eff[:], s_sum[:], -1.0, None, op0=Alu.add)
            nc.vector.tensor_mul(s_eff[:], s_eff[:], mask[:])
            nc.vector.tensor_scalar(s_eff[:], s_eff[:], 1.0, None, op0=Alu.add)
            # rs_mask = mask / s_eff ; c_log = ln(s_eff) ; csum = sum_t c_log
            rs_mask = small.tile([B, S], f32, tag="rs_mask")
            nc.vector.reciprocal(rs_mask[:], s_eff[:])
            nc.vector.tensor_mul(rs_mask[:], rs_mask[:], mask[:])
            c_log = small.tile([B, S], f32, tag="c_log")
            nc.scalar.activation(c_log[:], s_eff[:], Ln)
            csum = small.tile([B, 1], f32, tag="csum")
            nc.vector.tensor_reduce(csum[:], c_log[:], axis=Ax.X, op=Alu.add)
            # normalize
            for ck in range(S // TCK):
                sl = slice(ck * TCK, (ck + 1) * TCK)
                nc.vector.tensor_mul(
                    preT3[:, sl, :],
                    preT3[:, sl, :],
                    rs_mask[:, sl].unsqueeze(2).broadcast_to([B, TCK, J]),
                )

            # ---------------- transpose to scan space ----------------
            idn_i = small.tile([B, B], i32, tag="idn_i")
            nc.gpsimd.iota(idn_i[:], pattern=[[1, B]], base=0,
                           channel_multiplier=-1)
            idn = small.tile([B, B], f32, tag="idn")
            nc.vector.tensor_scalar(idn[:], idn_i[:], 0.0, None,
                                    op0=Alu.is_equal)
            idn128_i = small.tile([P, P], i32, tag="idn128_i")
            nc.gpsimd.iota(idn128_i[:], pattern=[[1, P]], base=0,
                           channel_multiplier=-1)
            idn128 = small.tile([P, P], f32, tag="idn128")
            nc.vector.tensor_scalar(idn128[:], idn128_i[:], 0.0, None,
                                    op0=Alu.is_equal)

            # escan[128, chunk, b]: chunk ck holds t=4ck..4ck+3 at row
            # offsets 0/32/64/96
            escan = big.tile([P, NCHUNK * B], f32, tag="escan")
            escan3 = escan[:].rearrange("p (n b) -> p n b", b=B)
            for ck in range(NCHUNK):
                t0 = ck * CHUNK_T
                src = preT3[:, t0 : t0 + CHUNK_T, :].rearrange(
                    "b s j -> b (s j)"
                )
                pt = ptrans.tile([P, B], f32, tag="pt")
                nc.tensor.matmul(pt[:], src, idn[:], is_transpose=True,
                                 start=True, stop=True)
                nc.scalar.copy(escan3[:, ck, :], pt[:])

            # omega row in (t, b) free layout, host-precomputed
            oslab = big.tile([1, S * B], f32, tag="oslab")
            nc.sync.dma_start(oslab[:], om_d.ap())

            # ---------------- weights W [JA, JA] (lhsT layout) -------------
            W = small.tile([JA, JA], f32, tag="W")
            nc.vector.memset(W[:], 0.0)
            nc.sync.dma_start(W[:J, :J], T_d.ap())
            nc.sync.dma_start(W[:J, J : J + 1], en_d.ap())
            nc.scalar.activation(W[:J, :], W[:J, :], Exp)
            nc.vector.memset(W[J : J + 1, J : J + 1], 1.0)

            # expStart [J, 1]
            est = small.tile([J, 1], f32, tag="est")
            nc.sync.dma_start(est[:], st_d.ap())
            nc.scalar.activation(est[:], est[:], Exp)
            # endp [JA, 1]: exp(end) rows 0..31, omega 1
            enp = small.tile([JA, 1], f32, tag="enp")
            nc.sync.dma_start(enp[:J, :], en_d.ap())
            nc.scalar.activation(enp[:J, :], enp[:J, :], Exp)
            nc.vector.memset(enp[J : J + 1, :], 1.0)
            ones_ja = small.tile([JA, 1], f32, tag="ones_ja")
            nc.vector.memset(ones_ja[:], 1.0)
            ones_b = small.tile([B, 1], f32, tag="ones_b")
            nc.vector.memset(ones_b[:], 1.0)
            ones_p = small.tile([P, 1], f32, tag="ones_p")
            nc.vector.memset(ones_p[:], 1.0)

            # ---------------- the scan ----------------
            def e_ap(t):
                return escan3[J * (t % CHUNK_T) : J * (t % CHUNK_T) + J,
                              t // CHUNK_T, :]

            def o_ap(t):
                return oslab[0:1, t * B : (t + 1) * B]

            a_prev = apool.tile([JA, B], f32, tag="a")
            nc.vector.tensor_scalar(
                a_prev[:J, :], e_ap(0), est[:], None, op0=Alu.mult
            )
            nc.vector.memset(a_prev[J : J + 1, :], 0.0)
            for t in range(1, scan_steps):
                ps = pscan.tile([JA, B], f32, tag="ps")
                nc.tensor.matmul(ps[:], W[:], a_prev[:], start=True, stop=True)
                a_t = apool.tile([JA, B], f32, tag="a")
                nc.vector.tensor_mul(a_t[:J, :], ps[:J, :], e_ap(t))
                nc.vector.tensor_mul(a_t[J : J + 1, :], ps[J : J + 1, :],
                                     o_ap(t))
                a_prev = a_t

            # ---------------- finalize log-partition ----------------
            af = apool.tile([JA, B], f32, tag="af")
            nc.vector.tensor_scalar(af[:], a_prev[:], enp[:], None,
                                    op0=Alu.mult)
            zrow = pfin.tile([1, B], f32, tag="fin_a")
            nc.tensor.matmul(zrow[:], ones_ja[:], af[:], start=True, stop=True)
            logz = small.tile([1, B], f32, tag="logz")
            nc.scalar.activation(logz[:], zrow[:], Ln)

            if with_score:
                # ============ log-score (batch-summed, no gathers) ============
                # [128, 64] relayout: partition p covers b = p//8,
                # t in [(p%8)*64, (p%8)*64+64)
                em128 = big.tile([P, FS * J], f32, tag="em128")
                nc.sync.dma_start(
                    em128[:], em_d.ap().rearrange("b s j -> (b s j)")
                    .rearrange("(p f) -> p f", p=P)
                )
                tgt128 = small.tile([P, FS], i32, tag="tgt128")
                nc.sync.dma_start(
                    tgt128[:], tgt_d.ap().rearrange("b s -> (b s)")
                    .rearrange("(p f) -> p f", p=P)
                )
                tgt128f = small.tile([P, FS], f32, tag="tgt128f")
                nc.vector.tensor_copy(tgt128f[:], tgt128[:])
                # shifted targets: tgt[b, t+1] at (p, f); last element garbage
                # but always masked (t=511 pair is never valid)
                tgtn128 = small.tile([P, FS], i32, tag="tgtn128")
                tgt_flat = tgt_d.ap().rearrange("b s -> (b s)")
                tgtv = tgt_flat.rearrange("(p f) -> p f", p=P)
                nc.vector.memset(tgtn128[:, FS - 1 : FS], 0)
                nc.sync.dma_start(tgtn128[:, : FS - 1], tgtv[:, 1:])
                nc.sync.dma_start(tgtn128[: P - 1, FS - 1 : FS], tgtv[1:, 0:1])
                tgtn128f = small.tile([P, FS], f32, tag="tgtn128f")
                nc.vector.tensor_copy(tgtn128f[:], tgtn128[:])
                # masks reshaped via SBUF->SBUF DMA
                # t-index and length in the [128, 64] layout, computed on-chip
                i64 = small.tile([P, FS], i32, tag="i64")
                nc.gpsimd.iota(i64[:], pattern=[[1, FS]], base=0,
                               channel_multiplier=FS)
                piota = small.tile([P, 1], i32, tag="piota")
                nc.gpsimd.iota(piota[:], pattern=[[0, 1]], base=0,
                               channel_multiplier=1)
                bq = small.tile([P, 1], i32, tag="bq")
                nc.vector.tensor_scalar(bq[:], piota[:], 3, None,
                                        op0=Alu.arith_shift_right)
                boff = small.tile([P, 1], i32, tag="boff")
                nc.vector.tensor_scalar(boff[:], bq[:], 9, None,
                                        op0=Alu.logical_shift_left)
                bofff = small.tile([P, 1], f32, tag="bofff")
                nc.vector.tensor_copy(bofff[:], boff[:])
                tv128 = small.tile([P, FS], f32, tag="tv128")
                nc.vector.tensor_copy(tv128[:], i64[:])
                nc.vector.tensor_scalar(tv128[:], tv128[:], bofff[:], None,
                                        op0=Alu.subtract)
                # len128[p] = len[p//8] via a one-hot matmul broadcast
                bqf = small.tile([P, 1], f32, tag="bqf")
                nc.vector.tensor_copy(bqf[:], bq[:])
                iota16 = small.tile([P, B], i32, tag="iota16")
                nc.gpsimd.iota(iota16[:], pattern=[[1, B]], base=0,
                               channel_multiplier=0)
                iota16f = small.tile([P, B], f32, tag="iota16f")
                nc.vector.tensor_copy(iota16f[:], iota16[:])
                b8t = small.tile([P, B], f32, tag="b8t")
                nc.vector.tensor_scalar(b8t[:], iota16f[:], bqf[:], None,
                                        op0=Alu.is_equal)
                pb8 = ptrans.tile([B, P], f32, tag="pt")
                nc.tensor.matmul(pb8[:], b8t[:], idn128[:], is_transpose=True,
                                 start=True, stop=True)
                b8 = small.tile([B, P], f32, tag="b8")
                nc.scalar.copy(b8[:], pb8[:])
                pl128 = pfin.tile([P, 1], f32, tag="fin_a")
                nc.tensor.matmul(pl128[:], b8[:], len_f[:], start=True, stop=True)
                len128 = small.tile([P, 1], f32, tag="len128")
                nc.scalar.copy(len128[:], pl128[:])
                len128m1 = small.tile([P, 1], f32, tag="len128m1")
                nc.vector.tensor_scalar(len128m1[:], len128[:], -1.0, None,
                                        op0=Alu.add)
                mask128 = small.tile([P, FS], f32, tag="mask128")
                nc.vector.tensor_scalar(mask128[:], tv128[:], len128[:], None,
                                        op0=Alu.is_lt)
                maskn128 = small.tile([P, FS], f32, tag="maskn128")
                nc.vector.tensor_scalar(maskn128[:], tv128[:], len128m1[:], None,
                                        op0=Alu.is_lt)
                last128 = small.tile([P, FS], f32, tag="last128")
                nc.vector.tensor_scalar(last128[:], tv128[:], len128m1[:], None,
                                        op0=Alu.is_equal)
                # masked target codes: tgt where valid else -1
                tgtmP = small.tile([P, FS], f32, tag="tgtmP")
                nc.vector.tensor_scalar(tgtmP[:], tgt128f[:], 1.0, None,
                                        op0=Alu.add)
                nc.vector.tensor_mul(tgtmP[:], tgtmP[:], mask128[:])
                nc.vector.tensor_scalar(tgtmP[:], tgtmP[:], -1.0, None,
                                        op0=Alu.add)
                tgtmN = small.tile([P, FS], f32, tag="tgtmN")
                nc.vector.tensor_scalar(tgtmN[:], tgtn128f[:], 1.0, None,
                                        op0=Alu.add)
                nc.vector.tensor_mul(tgtmN[:], tgtmN[:], maskn128[:])
                nc.vector.tensor_scalar(tgtmN[:], tgtmN[:], -1.0, None,
                                        op0=Alu.add)
                # one-hot slabs [P, FS, J] via small iota broadcast along f
                iota_ji = small.tile([P, J], i32, tag="iota_ji")
                nc.gpsimd.iota(iota_ji[:], pattern=[[1, J]], base=0,
                               channel_multiplier=0)
                iota_jf = small.tile([P, J], f32, tag="iota_jf")
                nc.vector.tensor_copy(iota_jf[:], iota_ji[:])
                iota_b = (iota_jf[:].unsqueeze(1)
                          .broadcast_to([P, FS, J]))
                ohp = big.tile([P, FS * J], f32, tag="ohp")
                nc.vector.tensor_tensor(
                    ohp[:].rearrange("p (f j) -> p f j", j=J),
                    iota_b,
                    tgtmP[:].unsqueeze(2).broadcast_to([P, FS, J]),
                    op=Alu.is_equal,
                )
                ohn = big.tile([P, FS * J], f32, tag="ohn")
                nc.vector.tensor_tensor(
                    ohn[:].rearrange("p (f j) -> p f j", j=J),
                    iota_b,
                    tgtmN[:].unsqueeze(2).broadcast_to([P, FS, J]),
                    op=Alu.is_equal,
                )
                # transition count matrix C[i,j] over all valid pairs
                ohp3 = ohp[:].rearrange("p (f j) -> p f j", j=J)
                ohn3 = ohn[:].rearrange("p (f j) -> p f j", j=J)
                cpsum = pfin.tile([J, J], f32, tag="fin_a")
                with tc.tile_critical():
                    for f in range(FS):
                        nc.tensor.matmul(cpsum[:], ohp3[:, f, :], ohn3[:, f, :],
                                         start=(f == 0), stop=(f == FS - 1))
                Traw = small.tile([J, J], f32, tag="Traw")
                nc.sync.dma_start(Traw[:], T_d.ap())
                tsc = small.tile([J, 1], f32, tag="tsc")
                tscratch = small.tile([J, J], f32, tag="tscratch")
                nc.vector.tensor_mul(tscratch[:], cpsum[:], Traw[:])
                nc.vector.tensor_reduce(tsc[:], tscratch[:], axis=Ax.X, op=Alu.add)
                # end term: weights = sum over (p,f) of ohp * last128 -> [J]
                # (reuse ohn slab slot is not needed; overwrite ohn in place)
                wsel = ohn  # reuse the ohn slab after the C matmuls consumed it
                nc.vector.tensor_tensor(
                    wsel[:].rearrange("p (f j) -> p f j", j=J),
                    ohp3,
                    last128[:].unsqueeze(2).broadcast_to([P, FS, J]),
                    op=Alu.mult,
                )
                wselred = small.tile([P, J], f32, tag="wselred")
                nc.vector.tensor_reduce(
                    wselred[:],
                    wsel[:].rearrange("p (f j) -> p j f", j=J),
                    axis=Ax.X, op=Alu.add,
                )
                endcnt = pfin.tile([J, 1], f32, tag="fin_a")
                nc.tensor.matmul(endcnt[:], wselred[:], ones_p[:], start=True,
                                 stop=True)
                en_raw = small.tile([J, 1], f32, tag="en_raw")
                nc.sync.dma_start(en_raw[:], en_d.ap())
                endsc = small.tile([J, 1], f32, tag="endsc")
                nc.vector.tensor_mul(endsc[:], endcnt[:], en_raw[:])
                # emission term: sum(ohp * em128) -- in-place over ohp
                nc.vector.tensor_mul(ohp[:], ohp[:], em128[:])
                emred = small.tile([P, 1], f32, tag="emred")
                nc.vector.tensor_reduce(emred[:], ohp[:], axis=Ax.X, op=Alu.add)
                emtot = pfin.tile([1, 1], f32, tag="fin_a")
                nc.tensor.matmul(emtot[:], emred[:], ones_p[:], start=True,
                                 stop=True)
                # start term: counts of tgt[b, 0]
                tgt0 = small.tile([B, 1], i32, tag="tgt0")
                nc.sync.dma_start(tgt0[:], tgt_d.ap()[:, 0:1])
                tgt0f = small.tile([B, 1], f32, tag="tgt0f")
                nc.vector.tensor_copy(tgt0f[:], tgt0[:])
                iota_jb = small.tile([B, J], i32, tag="iota_jb")
                nc.gpsimd.iota(iota_jb[:], pattern=[[1, J]], base=0,
                               channel_multiplier=0)
                iota_jbf = small.tile([B, J], f32, tag="iota_jbf")
                nc.vector.tensor_copy(iota_jbf[:], iota_jb[:])
                oh0 = small.tile([B, J], f32, tag="oh0")
                nc.vector.tensor_scalar(oh0[:], iota_jbf[:], tgt0f[:], None,
                                        op0=Alu.is_equal)
                cnt0 = pfin.tile([J, 1], f32, tag="fin_a")
                nc.tensor.matmul(cnt0[:], oh0[:], ones_b[:], start=True, stop=True)
                st_raw = small.tile([J, 1], f32, tag="st_raw")
                nc.sync.dma_start(st_raw[:], st_d.ap())
                stsc = small.tile([J, 1], f32, tag="stsc")
                nc.vector.tensor_mul(stsc[:], cnt0[:], st_raw[:])

            # ---------------- combine ----------------
            # NLL = sum_b logz + sum_b csum - (emtot + sum(tsc+endsc+stsc))
            s_all = pfin.tile([1, 1], f32, tag="fin_a")
            nc.tensor.matmul(s_all[:], ones_b[:], csum[:], start=True,
                             stop=True)
            s2 = small.tile([1, 1], f32, tag="s2")
            nc.vector.tensor_reduce(s2[:], logz[:], axis=Ax.X, op=Alu.add)
            res = small.tile([1, 1], f32, tag="res")
            nc.vector.tensor_add(res[:], s_all[:], s2[:])
            if with_score:
                sneg = small.tile([J, 1], f32, tag="sneg")
                nc.vector.tensor_add(sneg[:], tsc[:], endsc[:])
                nc.vector.tensor_add(sneg[:], sneg[:], stsc[:])
                nc.vector.tensor_scalar(sneg[:], sneg[:], -1.0, None,
                                        op0=Alu.mult)
                ones_j = small.tile([J, 1], f32, tag="ones_j")
                nc.vector.memset(ones_j[:], 1.0)
                s3p = pfin.tile([1, 1], f32, tag="fin_a")
                nc.tensor.matmul(s3p[:], ones_j[:], sneg[:], start=True,
                                 stop=True)
                nc.vector.tensor_add(res[:], res[:], s3p[:])
                nc.vector.tensor_sub(res[:], res[:], emtot[:])
            nc.sync.dma_start(out_d.ap(), res[:])

    nc.compile()
    return nc


_NC_CACHE = None


def kernel(emission, length, target, transition, start_transition,
           end_transition):
    global _NC_CACHE
    from concourse.bass_utils import run_bass_kernel_spmd

    emission = np.ascontiguousarray(np.asarray(emission, np.float32))
    length = np.asarray(length).astype(np.int32).reshape(-1, 1)
    target = np.asarray(target).astype(np.int32)
    transition = np.ascontiguousarray(np.asarray(transition, np.float32))
    start = np.asarray(start_transition, np.float32).reshape(J, 1)
    end = np.asarray(end_transition, np.float32).reshape(J, 1)

    if _NC_CACHE is None:
        _NC_CACHE = build_bass()
    nc = _NC_CACHE

    tgrid = np.arange(S)[:, None]
    in_maps = []
    for c in range(NCORES):
        sl = slice(c * B, (c + 1) * B)
        om = (tgrid >= length[sl].reshape(1, B)).astype(np.float32)
        in_maps.append({
            "omega": np.ascontiguousarray(om.reshape(1, S * B)),
            "emission": np.ascontiguousarray(emission[sl]),
            "length": np.ascontiguousarray(length[sl]),
            "target": np.ascontiguousarray(target[sl]),
            "transition": transition,
            "start_transition": start,
            "end_transition": end,
        })

    r = run_bass_kernel_spmd(nc, in_maps, list(range(NCORES)))
    total = np.float64(0.0)
    for c in range(NCORES):
        total += np.float64(r.results[c]["out"][0, 0])
    return np.asarray(total, np.float32)


if __name__ == "__main__":
    rng = np.random.default_rng(0)
    inputs = {
        "emission": rng.standard_normal((128, S, J)).astype(np.float32),
        "length": rng.integers(2, S + 1, size=(128,)),
        "target": rng.integers(0, J, size=(128, S)),
        "transition": (rng.standard_normal((J, J)) * 0.1).astype(np.float32),
        "start_transition": (rng.standard_normal(J) * 0.1).astype(np.float32),
        "end_transition": (rng.standard_normal(J) * 0.1).astype(np.float32),
    }
    print(kernel(**inputs))



# revision 10
# speedup vs baseline: 2.7515x; 2.7515x over previous
"""CRF negative-log-likelihood kernel for Trainium2 (Bass/Tile), 8-core SPMD.

Problem: emission [128, 512, 32] f32, length [128], target [128, 512],
transition [32, 32], start/end_transition [32] -> scalar f32
  sum_b (log_partition_b - log_score_b)

Strategy (data-parallel over batch, 16 sequences per core):
  * log_partition via the forward algorithm in EXP space:
      A_t = E_t .* (W^T A_{t-1}),  E_t = exp(em_t) * mask / s_eff  (row 32 =
    absorbing "omega" tag carrying variable-length sequences; the dropped
    per-(t,b) scale ln(s_eff) is accumulated separately).
  * BIDIRECTIONAL: forward chain computes A_255 from t=0; an independent
    backward chain computes v_255 (v_t := transpose-product of the remaining
    steps applied to the end vector) from t=511 down:
      v_{t-1} = W' (E_t .* v_t),  v_511 = (exp(end); 1).
    Z_b = v_255^T A_255.  Halves the sequential chain to 256 steps.
  * Chain engines: PE does the 33x33 x 33x16 matmuls; GPSIMD (Pool) does the
    elementwise E multiplies (PSUM in, SBUF out).  DVE never touches the
    chain, so the whole log-score computation runs on DVE underneath it.
  * Prep in the full-partition [128, 2048] layout (p = 8b + (t>>6), free =
    (t&63)*32 + j): one contiguous DMA, exp/normalize at 128-way parallelism,
    then 22 wide PE transposes ([128,99] -> [99,128]) produce the scan-order
    E slabs (3 timesteps x 33 tags incl. omega per slab), left in PSUM.
  * log_score summed over batch with one-hot / count-matrix contractions in
    the same [128, 64] layout (no gathers); the 64 PSUM-accumulated count
    matmuls are interleaved into the chain loop as PE fillers.
  * Each core writes one partial sum; the host adds the 8 partials.
"""

import numpy as np

B = 16           # batch per core
S = 512          # sequence length
J = 32           # tags
JA = J + 1       # augmented with omega
NCORES = 8
P = 128          # partitions
SH = 8           # s_hi values (t >> 6)
SL = 64          # s_lo values (t & 63)
FS = SL          # free elems per partition in the [128, 64] layout
NQ = 32          # transpose slabs: 2 s_lo x 64-row blocks each
HF = 256         # meet point: fwd computes A_{HF-1}, bwd computes v_{HF-1}

ESC_IN_PSUM = False
CPSUM_INLINE = True


def build_bass(do_scan=True, do_score=True):
    import concourse.bacc as bacc
    import concourse.tile as tile
    from concourse import mybir

    f32 = mybir.dt.float32
    i32 = mybir.dt.int32

    nc = bacc.Bacc(
        "TRN2", target_bir_lowering=False, debug=False, num_devices=NCORES
    )

    em_d = nc.dram_tensor("emission", [B, S, J], f32, kind="ExternalInput")
    len_d = nc.dram_tensor("length", [B, 1], i32, kind="ExternalInput")
    tgt_d = nc.dram_tensor("target", [B, S], i32, kind="ExternalInput")
    T_d = nc.dram_tensor("transition", [J, J], f32, kind="ExternalInput")
    st_d = nc.dram_tensor("start_transition", [J, 1], f32, kind="ExternalInput")
    en_d = nc.dram_tensor("end_transition", [J, 1], f32, kind="ExternalInput")
    out_d = nc.dram_tensor("out", [1, 1], f32, kind="ExternalOutput")

    Exp = mybir.ActivationFunctionType.Exp
    Ln = mybir.ActivationFunctionType.Ln
    Alu = mybir.AluOpType
    Ax = mybir.AxisListType

    with tile.TileContext(nc) as tc:
        with (
            tc.tile_pool(name="big", bufs=1) as big,
            tc.tile_pool(name="small", bufs=1) as small,
            tc.tile_pool(name="apoolF", bufs=2) as apoolF,
            tc.tile_pool(name="apoolB", bufs=2) as apoolB,
            tc.tile_pool(name="pesc", bufs=2, space="PSUM") as pesc,
            tc.tile_pool(name="pscanF", bufs=2, space="PSUM") as pscanF,
            tc.tile_pool(name="pscanB", bufs=2, space="PSUM") as pscanB,
            tc.tile_pool(name="pcp", bufs=1, space="PSUM") as pcp,
            tc.tile_pool(name="pfin", bufs=1, space="PSUM") as pfin,
        ):
            # ================= loads =================
            em_raw = big.tile([P, SL * J], f32, tag="em_raw")
            nc.sync.dma_start(
                em_raw[:], em_d.ap().rearrange("b s j -> (b s j)")
                .rearrange("(p f) -> p f", p=P)
            )
            len_i = small.tile([B, 1], i32, tag="len_i")
            nc.sync.dma_start(len_i[:], len_d.ap())
            len_f = small.tile([B, 1], f32, tag="len_f")
            nc.vector.tensor_copy(len_f[:], len_i[:])

            # ================= index helpers =================
            # b8T[c, p] = (p >> 3 == c)  -> len128 = b8T^T @ len_f
            i128r = small.tile([B, P], i32, tag="i128r")
            nc.gpsimd.iota(i128r[:], pattern=[[1, P]], base=0,
                           channel_multiplier=0)
            i128rs = small.tile([B, P], i32, tag="i128rs")
            nc.vector.tensor_scalar(i128rs[:], i128r[:], 3, None,
                                    op0=Alu.arith_shift_right)
            i128rf = small.tile([B, P], f32, tag="i128rf")
            nc.vector.tensor_copy(i128rf[:], i128rs[:])
            c16 = small.tile([B, 1], i32, tag="c16")
            nc.gpsimd.iota(c16[:], pattern=[[0, 1]], base=0,
                           channel_multiplier=1)
            c16f = small.tile([B, 1], f32, tag="c16f")
            nc.vector.tensor_copy(c16f[:], c16[:])
            b8T = small.tile([B, P], f32, tag="b8T")
            nc.vector.tensor_scalar(b8T[:], i128rf[:], c16f[:], None,
                                    op0=Alu.is_equal)
            def fin_tile(n):
                t = pfin.tile([P, 1], f32, tag="fin")
                return t[0:n, :]

            pl128 = fin_tile(P)
            nc.tensor.matmul(pl128[:], b8T[:], len_f[:], start=True, stop=True)
            len128 = small.tile([P, 1], f32, tag="len128")
            nc.scalar.copy(len128[:], pl128[:])
            len128m1 = small.tile([P, 1], f32, tag="len128m1")
            nc.vector.tensor_scalar(len128m1[:], len128[:], -1.0, None,
                                    op0=Alu.add)

            # tv128[p, f] = t = (p & 7) * 64 + f
            i64 = small.tile([P, FS], i32, tag="i64")
            nc.gpsimd.iota(i64[:], pattern=[[1, FS]], base=0,
                           channel_multiplier=FS)
            piota = small.tile([P, 1], i32, tag="piota")
            nc.gpsimd.iota(piota[:], pattern=[[0, 1]], base=0,
                           channel_multiplier=1)
            bq = small.tile([P, 1], i32, tag="bq")
            nc.vector.tensor_scalar(bq[:], piota[:], 3, None,
                                    op0=Alu.arith_shift_right)
            boff = small.tile([P, 1], i32, tag="boff")
            nc.vector.tensor_scalar(boff[:], bq[:], 9, None,
                                    op0=Alu.logical_shift_left)
            bofff = small.tile([P, 1], f32, tag="bofff")
            nc.vector.tensor_copy(bofff[:], boff[:])
            tv128 = small.tile([P, FS], f32, tag="tv128")
            nc.vector.tensor_copy(tv128[:], i64[:])
            nc.vector.tensor_scalar(tv128[:], tv128[:], bofff[:], None,
                                    op0=Alu.subtract)

            mask128 = small.tile([P, FS], f32, tag="mask128")
            nc.vector.tensor_scalar(mask128[:], tv128[:], len128[:], None,
                                    op0=Alu.is_lt)
            omega = small.tile([P, FS], f32, tag="omega")
            nc.vector.tensor_scalar(omega[:], tv128[:], len128[:], None,
                                    op0=Alu.is_ge)

            # idn128 for the transposes
            idn_i = small.tile([P, P], i32, tag="idn_i")
            nc.gpsimd.iota(idn_i[:], pattern=[[1, P]], base=0,
                           channel_multiplier=-1)
            idn128 = small.tile([P, P], f32, tag="idn128")
            nc.vector.tensor_scalar(idn128[:], idn_i[:], 0.0, None,
                                    op0=Alu.is_equal)

            # ================= E slab: exp, stats, normalize =================
            E = big.tile([P, SL * SL], f32, tag="E")
            E3 = E[:].rearrange("p (s j) -> p s j", j=SL)
            em3 = em_raw[:].rearrange("p (s j) -> p s j", j=J)
            nc.scalar.activation(E3[:, :, :J], em3[:], Exp)
            s_sum = small.tile([P, FS], f32, tag="s_sum")
            nc.vector.tensor_reduce(s_sum[:], E3[:, :, :J], axis=Ax.X,
                                    op=Alu.add)
            # s_eff = (s_sum - 1) * mask + 1 ; rs = mask / s_eff
            s_eff = small.tile([P, FS], f32, tag="s_eff")
            nc.vector.tensor_scalar(s_eff[:], s_sum[:], -1.0, None, op0=Alu.add)
            nc.vector.tensor_mul(s_eff[:], s_eff[:], mask128[:])
            nc.vector.tensor_scalar(s_eff[:], s_eff[:], 1.0, None, op0=Alu.add)
            rs = small.tile([P, FS], f32, tag="rs")
            nc.vector.reciprocal(rs[:], s_eff[:])
            nc.vector.tensor_mul(rs[:], rs[:], mask128[:])
            c_log = small.tile([P, FS], f32, tag="c_log")
            nc.scalar.activation(c_log[:], s_eff[:], Ln)
            csum = small.tile([P, 1], f32, tag="csum")
            nc.vector.tensor_reduce(csum[:], c_log[:], axis=Ax.X, op=Alu.add)
            nc.vector.tensor_mul(
                E3[:, :, :J], E3[:, :, :J],
                rs[:].unsqueeze(2).broadcast_to([P, FS, J]),
            )
            nc.vector.tensor_copy(E3[:, :, J:JA], omega[:].unsqueeze(2))

            ones_p = small.tile([P, 1], f32, tag="ones_p")
            nc.vector.memset(ones_p[:], 1.0)
            s_all = small.tile([1, 1], f32, tag="s_all")
            nc.gpsimd.tensor_reduce(s_all[:], csum[:], axis=Ax.XYZWC,
                                    op=Alu.add)

            # ================= weights & seeds =================
            W = small.tile([JA, JA], f32, tag="W")
            nc.vector.memset(W[:], 0.0)
            nc.sync.dma_start(W[:J, :J], T_d.ap())
            nc.sync.dma_start(W[:J, J : J + 1], en_d.ap())
            nc.scalar.activation(W[:J, :], W[:J, :], Exp)
            nc.vector.memset(W[J : J + 1, J : J + 1], 1.0)

            Wt = small.tile([JA, JA], f32, tag="Wt")
            nc.vector.memset(Wt[:], 0.0)
            nc.sync.dma_start(Wt[:J, :J], T_d.ap().rearrange("i j -> j i"))
            nc.sync.dma_start(Wt[J : J + 1, :J],
                              en_d.ap().rearrange("i j -> j i"))
            nc.scalar.activation(Wt[:, :J], Wt[:, :J], Exp)
            nc.vector.memset(Wt[J : J + 1, J : J + 1], 1.0)

            est_aug = small.tile([JA, 1], f32, tag="est_aug")
            nc.vector.memset(est_aug[:], 0.0)
            nc.sync.dma_start(est_aug[:J, :], st_d.ap())
            nc.scalar.activation(est_aug[:J, :], est_aug[:J, :], Exp)
            enp = small.tile([JA, 1], f32, tag="enp")
            nc.vector.memset(enp[:], 1.0)
            nc.sync.dma_start(enp[:J, :], en_d.ap())
            nc.scalar.activation(enp[:J, :], enp[:J, :], Exp)

            ones_ja = small.tile([JA, 1], f32, tag="ones_ja")
            nc.vector.memset(ones_ja[:], 1.0)
            ones_b = small.tile([B, 1], f32, tag="ones_b")
            nc.vector.memset(ones_b[:], 1.0)
            ones_j = small.tile([J, 1], f32, tag="ones_j")
            nc.vector.memset(ones_j[:], 1.0)

            # ================= transposes -> scan-order E =================
            escs = big.tile([P, NQ * P], f32, tag="escs")

            def emit_transpose(q):
                src = E[:, P * q : P * q + P]
                pt = pesc.tile([P, P], f32, tag="pt")
                nc.tensor.matmul(pt[:], src, idn128[:],
                                 is_transpose=True, start=True, stop=True)
                nc.scalar.copy(escs[:, q * P : (q + 1) * P], pt[:])
                return None

            qorder = []
            lo, hi = 0, NQ - 1
            while lo <= hi:
                qorder.append(lo)
                if hi != lo:
                    qorder.append(hi)
                lo += 1
                hi -= 1
            tiles = {}
            for q in qorder:
                tiles[q] = emit_transpose(q)

            def e_ap(t):
                sl_, sh_ = t & 63, t >> 6
                q, r = sl_ >> 1, sl_ & 1
                v = escs[:, q * P : (q + 1) * P].rearrange(
                    "p (b s) -> p s b", b=B)
                return v[64 * r : 64 * r + JA, sh_, :]

            # ================= score prep (DVE only + few PE) ==============
            if do_score:
                maskn128 = small.tile([P, FS], f32, tag="maskn128")
                nc.vector.tensor_scalar(maskn128[:], tv128[:], len128m1[:],
                                        None, op0=Alu.is_lt)
                last128 = small.tile([P, FS], f32, tag="last128")
                nc.vector.tensor_scalar(last128[:], tv128[:], len128m1[:],
                                        None, op0=Alu.is_equal)
                tgt128 = small.tile([P, FS], i32, tag="tgt128")
                nc.sync.dma_start(
                    tgt128[:], tgt_d.ap().rearrange("b s -> (b s)")
                    .rearrange("(p f) -> p f", p=P)
                )
                tgt128f = small.tile([P, FS], f32, tag="tgt128f")
                nc.vector.tensor_copy(tgt128f[:], tgt128[:])
                tgtn128 = small.tile([P, FS], i32, tag="tgtn128")
                tgt_flat = tgt_d.ap().rearrange("b s -> (b s)")
                tgtv = tgt_flat.rearrange("(p f) -> p f", p=P)
                nc.vector.memset(tgtn128[:, FS - 1 : FS], 0)
                nc.sync.dma_start(tgtn128[:, : FS - 1], tgtv[:, 1:])
                nc.sync.dma_start(tgtn128[: P - 1, FS - 1 : FS], tgtv[1:, 0:1])
                tgtn128f = small.tile([P, FS], f32, tag="tgtn128f")
                nc.vector.tensor_copy(tgtn128f[:], tgtn128[:])
                # masked codes: tgt where valid else -1
                tgtmP = small.tile([P, FS], f32, tag="tgtmP")
                nc.vector.tensor_scalar(tgtmP[:], tgt128f[:], 1.0, None,
                                        op0=Alu.add)
                nc.vector.tensor_mul(tgtmP[:], tgtmP[:], mask128[:])
                nc.vector.tensor_scalar(tgtmP[:], tgtmP[:], -1.0, None,
                                        op0=Alu.add)
                tgtmN = small.tile([P, FS], f32, tag="tgtmN")
                nc.vector.tensor_scalar(tgtmN[:], tgtn128f[:], 1.0, None,
                                        op0=Alu.add)
                nc.vector.tensor_mul(tgtmN[:], tgtmN[:], maskn128[:])
                nc.vector.tensor_scalar(tgtmN[:], tgtmN[:], -1.0, None,
                                        op0=Alu.add)
                iota_ji = small.tile([P, J], i32, tag="iota_ji")
                nc.gpsimd.iota(iota_ji[:], pattern=[[1, J]], base=0,
                               channel_multiplier=0)
                iota_jf = small.tile([P, J], f32, tag="iota_jf")
                nc.vector.tensor_copy(iota_jf[:], iota_ji[:])
                iota_b = iota_jf[:].unsqueeze(1).broadcast_to([P, FS, J])
                ohp = big.tile([P, FS * J], f32, tag="ohp")
                ohp3 = ohp[:].rearrange("p (f j) -> p f j", j=J)
                nc.vector.tensor_tensor(
                    ohp3, iota_b,
                    tgtmP[:].unsqueeze(2).broadcast_to([P, FS, J]),
                    op=Alu.is_equal,
                )
                ohn = big.tile([P, FS * J], f32, tag="ohn")
                ohn3 = ohn[:].rearrange("p (f j) -> p f j", j=J)
                nc.vector.tensor_tensor(
                    ohn3, iota_b,
                    tgtmN[:].unsqueeze(2).broadcast_to([P, FS, J]),
                    op=Alu.is_equal,
                )
                # end-term selector and emission product (separate slabs so
                # ohp/ohn stay pristine for the interleaved count matmuls)
                endJ = small.tile([P, J], f32, tag="endJ")
                nc.sync.dma_start(
                    endJ[:],
                    en_d.ap().rearrange("j one -> (one j)").unsqueeze(0)
                    .broadcast_to([P, J]),
                )
                wsel = big.tile([P, FS * J], f32, tag="wsel")
                nc.gpsimd.tensor_tensor(
                    wsel[:].rearrange("p (f j) -> p f j", j=J), ohp3,
                    last128[:].unsqueeze(2).broadcast_to([P, FS, J]),
                    op=Alu.mult,
                )
                nc.gpsimd.tensor_tensor(
                    wsel[:].rearrange("p (f j) -> p f j", j=J),
                    wsel[:].rearrange("p (f j) -> p f j", j=J),
                    endJ[:].unsqueeze(1).broadcast_to([P, FS, J]),
                    op=Alu.mult,
                )
                endtot_s = small.tile([1, 1], f32, tag="endtot_s")
                nc.gpsimd.tensor_reduce(endtot_s[:], wsel[:], axis=Ax.XYZWC,
                                        op=Alu.add)
                prod = big.tile([P, FS * J], f32, tag="prod")
                nc.gpsimd.tensor_mul(prod[:], ohp[:], em_raw[:])
                emtot_s = small.tile([1, 1], f32, tag="emtot_s")
                nc.gpsimd.tensor_reduce(emtot_s[:], prod[:], axis=Ax.XYZWC,
                                        op=Alu.add)
                # start term one-hot
                tgt0 = small.tile([B, 1], i32, tag="tgt0")
                nc.sync.dma_start(tgt0[:], tgt_d.ap()[:, 0:1])
                tgt0f = small.tile([B, 1], f32, tag="tgt0f")
                nc.vector.tensor_copy(tgt0f[:], tgt0[:])
                iota_jb = small.tile([B, J], i32, tag="iota_jb")
                nc.gpsimd.iota(iota_jb[:], pattern=[[1, J]], base=0,
                               channel_multiplier=0)
                iota_jbf = small.tile([B, J], f32, tag="iota_jbf")
                nc.vector.tensor_copy(iota_jbf[:], iota_jb[:])
                oh0 = small.tile([B, J], f32, tag="oh0")
                nc.vector.tensor_scalar(oh0[:], iota_jbf[:], tgt0f[:], None,
                                        op0=Alu.is_equal)
                Traw = small.tile([J, J], f32, tag="Traw")
                nc.sync.dma_start(Traw[:], T_d.ap())
                st_raw = small.tile([J, 1], f32, tag="st_raw")
                nc.sync.dma_start(st_raw[:], st_d.ap())
                cpsum = pcp.tile([J, J], f32, tag="cpsum")

            # ================= the bidirectional chain =================
            if do_scan:
                aF = apoolF.tile([JA, B], f32, tag="aF")
                nc.vector.tensor_scalar(aF[:], e_ap(0), est_aug[:], None,
                                        op0=Alu.mult)
                xB = apoolB.tile([JA, B], f32, tag="xB")
                nc.vector.tensor_scalar(xB[:], e_ap(2 * HF - 1), enp[:], None,
                                        op0=Alu.mult)
                psB = pscanB.tile([JA, B], f32, tag="psB")
                nc.tensor.matmul(psB[:], Wt[:], xB[:], start=True, stop=True)

                cp_sched = {}
                if do_score and CPSUM_INLINE:
                    for f in range(FS):
                        cp_sched[60 + 3 * f] = f

                for k in range(1, HF):
                    psF = pscanF.tile([JA, B], f32, tag="psF")
                    nc.tensor.matmul(psF[:], W[:], aF[:], start=True,
                                     stop=True)
                    xB2 = apoolB.tile([JA, B], f32, tag="xB")
                    nc.vector.tensor_mul(xB2[:], psB[:], e_ap(2 * HF - 1 - k))
                    aF2 = apoolF.tile([JA, B], f32, tag="aF")
                    nc.vector.tensor_mul(aF2[:], psF[:], e_ap(k))
                    psB = pscanB.tile([JA, B], f32, tag="psB")
                    nc.tensor.matmul(psB[:], Wt[:], xB2[:], start=True,
                                     stop=True)
                    aF, xB = aF2, xB2
                    if k in cp_sched:
                        f = cp_sched[k]
                        nc.tensor.matmul(cpsum[:], ohp3[:, f, :],
                                         ohn3[:, f, :], start=(f == 0),
                                         stop=(f == FS - 1))

                if do_score and not CPSUM_INLINE:
                    for f in range(FS):
                        nc.tensor.matmul(cpsum[:], ohp3[:, f, :],
                                         ohn3[:, f, :], start=(f == 0),
                                         stop=(f == FS - 1))

                # ---- meet: Z_b = v_255^T A_255 ----
                rm = apoolF.tile([JA, B], f32, tag="rm")
                nc.vector.tensor_mul(rm[:], psB[:], aF[:])
                zrow_t = pscanF.tile([JA, B], f32, tag="psF")
                zrow = zrow_t[0:1, :]
                nc.tensor.matmul(zrow, ones_ja[:], rm[:], start=True,
                                 stop=True)
                logz = small.tile([1, B], f32, tag="logz")
                nc.scalar.activation(logz[:], zrow, Ln)

            # ================= combine =================
            res = small.tile([1, 1], f32, tag="res")
            if do_scan:
                s2 = small.tile([1, 1], f32, tag="s2")
                nc.vector.tensor_reduce(s2[:], logz[:], axis=Ax.X, op=Alu.add)
                nc.vector.tensor_add(res[:], s_all[:], s2[:])
            else:
                nc.vector.tensor_copy(res[:], s_all[:])

            if do_score:
                # pfin has bufs=1: each new tile reuses the same PSUM bank, so
                # every tile's reads are emitted before the next tile call.
                cnt0 = fin_tile(J)
                nc.tensor.matmul(cnt0[:], oh0[:], ones_b[:], start=True,
                                 stop=True)
                stsc = small.tile([J, 1], f32, tag="stsc")
                nc.vector.tensor_mul(stsc[:], cnt0[:], st_raw[:])
                nc.vector.tensor_sub(res[:], res[:], emtot_s[:])
                nc.vector.tensor_sub(res[:], res[:], endtot_s[:])
                tscratch = small.tile([J, J], f32, tag="tscratch")
                nc.vector.tensor_mul(tscratch[:], cpsum[:], Traw[:])
                tsc = small.tile([J, 1], f32, tag="tsc")
                nc.vector.tensor_reduce(tsc[:], tscratch[:], axis=Ax.X,
                                        op=Alu.add)
                sneg = small.tile([J, 1], f32, tag="sneg")
                nc.vector.tensor_add(sneg[:], tsc[:], stsc[:])
                nc.vector.tensor_scalar(sneg[:], sneg[:], -1.0, None,
                                        op0=Alu.mult)
                s3p = fin_tile(1)
                nc.tensor.matmul(s3p[:], ones_j[:], sneg[:], start=True,
                                 stop=True)
                nc.vector.tensor_add(res[:], res[:], s3p[:])
            nc.sync.dma_start(out_d.ap(), res[:])

    nc.compile()
    return nc


_NC_CACHE = None


def kernel(emission, length, target, transition, start_transition,
           end_transition):
    global _NC_CACHE
    from concourse.bass_utils import run_bass_kernel_spmd

    emission = np.ascontiguousarray(np.asarray(emission, np.float32))
    length = np.asarray(length).astype(np.int32).reshape(-1, 1)
    target = np.asarray(target).astype(np.int32)
    transition = np.ascontiguousarray(np.asarray(transition, np.float32))
    start = np.asarray(start_transition, np.float32).reshape(J, 1)
    end = np.asarray(end_transition, np.float32).reshape(J, 1)

    if _NC_CACHE is None:
        _NC_CACHE = build_bass()
    nc = _NC_CACHE

    in_maps = []
    for c in range(NCORES):
        sl = slice(c * B, (c + 1) * B)
        in_maps.append({
            "emission": np.ascontiguousarray(emission[sl]),
            "length": np.ascontiguousarray(length[sl]),
            "target": np.ascontiguousarray(target[sl]),
            "transition": transition,
            "start_transition": start,
            "end_transition": end,
        })

    r = run_bass_kernel_spmd(nc, in_maps, list(range(NCORES)))
    total = np.float64(0.0)
    for c in range(NCORES):
        total += np.float64(r.results[c]["out"][0, 0])
    return np.asarray(total, np.float32)


if __name__ == "__main__":
    rng = np.random.default_rng(0)
    inputs = {
        "emission": rng.standard_normal((128, S, J)).astype(np.float32),
        "length": rng.integers(2, S + 1, size=(128,)),
        "target": rng.integers(0, J, size=(128, S)),
        "transition": (rng.standard_normal((J, J)) * 0.1).astype(np.float32),
        "start_transition": (rng.standard_normal(J) * 0.1).astype(np.float32),
        "end_transition": (rng.standard_normal(J) * 0.1).astype(np.float32),
    }
    print(kernel(**inputs))


# revision 18
# speedup vs baseline: 2.7939x; 1.0154x over previous
"""CRF negative-log-likelihood kernel for Trainium2 (Bass/Tile), 8-core SPMD.

Problem: emission [128, 512, 32] f32, length [128], target [128, 512],
transition [32, 32], start/end_transition [32] -> scalar f32
  sum_b (log_partition_b - log_score_b)

Strategy (data-parallel over batch, 16 sequences per core):
  * log_partition via the forward algorithm in EXP space:
      A_t = E_t .* (W^T A_{t-1}),  E_t = exp(em_t) * mask / s_eff  (row 32 =
    absorbing "omega" tag carrying variable-length sequences; the dropped
    per-(t,b) scale ln(s_eff) is accumulated separately).
  * BIDIRECTIONAL: forward chain computes A_255 from t=0; an independent
    backward chain computes v_255 (v_t := transpose-product of the remaining
    steps applied to the end vector) from t=511 down:
      v_{t-1} = W' (E_t .* v_t),  v_511 = (exp(end); 1).
    Z_b = v_255^T A_255.  Halves the sequential chain to 256 steps.
  * Chain engines: PE does the 33x33 x 33x16 matmuls; DVE does the
    elementwise E multiplies (PSUM in, SBUF out).  Everything else is kept
    off the chain's in-order engine streams or interleaved into its idle
    windows.
  * Prep in the full-partition [128, 2048] layout (p = 8b + (t>>6), free =
    (t&63)*32 + j), pipelined in 4 column chunks: DMA -> exp/normalize ->
    eight [128,128] PE transposes per chunk produce the scan-order E slabs
    (2 timesteps x 64-row-aligned blocks, omega at row offset 32).
  * log_score summed over batch with one-hot / count-matrix contractions in
    the same [128, 64] layout (no gathers).  Small DVE ops and the one-hot
    builds are sliced into the chain loop; the big elementwise multiplies and
    full reductions run on GPSIMD (SBUF-only there); the 64 PSUM-accumulated
    count matmuls are interleaved into the chain loop as PE fillers.
  * Each core writes one partial sum; the host adds the 8 partials.
"""

import numpy as np

B = 16           # batch per core
S = 512          # sequence length
J = 32           # tags
JA = J + 1       # augmented with omega
NCORES = 8
P = 128          # partitions
SH = 8           # s_hi values (t >> 6)
SL = 64          # s_lo values (t & 63)
FS = SL          # free elems per partition in the [128, 64] layout
NQ = 32          # transpose slabs: 2 s_lo x 64-row blocks each
HF = 256         # meet point: fwd computes A_{HF-1}, bwd computes v_{HF-1}
CPSUM_INLINE = True


def build_bass(do_scan=True, do_score=True):
    import concourse.bacc as bacc
    import concourse.tile as tile
    from concourse import mybir

    f32 = mybir.dt.float32
    i32 = mybir.dt.int32

    nc = bacc.Bacc(
        "TRN2", target_bir_lowering=False, debug=False, num_devices=NCORES
    )

    em_d = nc.dram_tensor("emission", [B, S, J], f32, kind="ExternalInput")
    len_d = nc.dram_tensor("length", [B, 1], i32, kind="ExternalInput")
    tgt_d = nc.dram_tensor("target", [B, S], i32, kind="ExternalInput")
    T_d = nc.dram_tensor("transition", [J, J], f32, kind="ExternalInput")
    st_d = nc.dram_tensor("start_transition", [J, 1], f32, kind="ExternalInput")
    en_d = nc.dram_tensor("end_transition", [J, 1], f32, kind="ExternalInput")
    out_d = nc.dram_tensor("out", [1, 1], f32, kind="ExternalOutput")

    Exp = mybir.ActivationFunctionType.Exp
    Ln = mybir.ActivationFunctionType.Ln
    Alu = mybir.AluOpType
    Ax = mybir.AxisListType

    CORD = [0, 3, 1, 2]  # chunk order: both chain heads first

    with tile.TileContext(nc) as tc:
        with (
            tc.tile_pool(name="big", bufs=1) as big,
            tc.tile_pool(name="small", bufs=1) as small,
            tc.tile_pool(name="apoolF", bufs=2) as apoolF,
            tc.tile_pool(name="apoolB", bufs=2) as apoolB,
            tc.tile_pool(name="pesc", bufs=2, space="PSUM") as pesc,
            tc.tile_pool(name="pscanF", bufs=2, space="PSUM") as pscanF,
            tc.tile_pool(name="pscanB", bufs=2, space="PSUM") as pscanB,
            tc.tile_pool(name="pcp", bufs=1, space="PSUM") as pcp,
            tc.tile_pool(name="pfin", bufs=1, space="PSUM") as pfin,
        ):
            # ======== DMAs, issued in critical-path order ========
            em_raw = big.tile([P, SL * J], f32, tag="em_raw")
            em_flat = (em_d.ap().rearrange("b s j -> (b s j)")
                       .rearrange("(p f) -> p f", p=P))
            SC = FS // 4  # 16 s_lo per chunk

            def em_dma(c):
                nc.sync.dma_start(em_raw[:, SC * J * c : SC * J * (c + 1)],
                                  em_flat[:, SC * J * c : SC * J * (c + 1)])

            len_i = small.tile([B, 1], i32, tag="len_i")
            nc.sync.dma_start(len_i[:], len_d.ap())
            em_dma(0)
            em_dma(3)
            Traw = small.tile([J, J], f32, tag="Traw")
            nc.sync.dma_start(Traw[:], T_d.ap())
            enC = small.tile([J, 1], f32, tag="enC")
            nc.sync.dma_start(enC[:], en_d.ap())
            em_dma(2)
            st_raw = small.tile([J, 1], f32, tag="st_raw")
            nc.sync.dma_start(st_raw[:], st_d.ap())
            em_dma(1)
            endJ = small.tile([P, J], f32, tag="endJ")
            nc.sync.dma_start(
                endJ[:],
                en_d.ap().rearrange("j one -> (one j)").unsqueeze(0)
                .broadcast_to([P, J]),
            )
            if do_score:
                tgt128 = small.tile([P, FS], i32, tag="tgt128")
                nc.sync.dma_start(
                    tgt128[:], tgt_d.ap().rearrange("b s -> (b s)")
                    .rearrange("(p f) -> p f", p=P)
                )
                tgtn128 = small.tile([P, FS], i32, tag="tgtn128")
                nc.vector.memset(tgtn128[:, FS - 1 : FS], 0)
                tgtv = (tgt_d.ap().rearrange("b s -> (b s)")
                        .rearrange("(p f) -> p f", p=P))
                nc.sync.dma_start(tgtn128[:, : FS - 1], tgtv[:, 1:])
                nc.sync.dma_start(tgtn128[: P - 1, FS - 1 : FS], tgtv[1:, 0:1])
                tgt0 = small.tile([B, 1], i32, tag="tgt0")
                nc.sync.dma_start(tgt0[:], tgt_d.ap()[:, 0:1])

            # ======== index helpers ========
            len_f = small.tile([B, 1], f32, tag="len_f")
            nc.vector.tensor_copy(len_f[:], len_i[:])
            # b8T[c, p] = (p >> 3 == c)  -> len128 = b8T^T @ len_f
            i128r = small.tile([B, P], i32, tag="i128r")
            nc.gpsimd.iota(i128r[:], pattern=[[1, P]], base=0,
                           channel_multiplier=0)
            i128rs = small.tile([B, P], i32, tag="i128rs")
            nc.vector.tensor_scalar(i128rs[:], i128r[:], 3, None,
                                    op0=Alu.arith_shift_right)
            i128rf = small.tile([B, P], f32, tag="i128rf")
            nc.vector.tensor_copy(i128rf[:], i128rs[:])
            c16 = small.tile([B, 1], i32, tag="c16")
            nc.gpsimd.iota(c16[:], pattern=[[0, 1]], base=0,
                           channel_multiplier=1)
            c16f = small.tile([B, 1], f32, tag="c16f")
            nc.vector.tensor_copy(c16f[:], c16[:])
            b8T = small.tile([B, P], f32, tag="b8T")
            nc.vector.tensor_scalar(b8T[:], i128rf[:], c16f[:], None,
                                    op0=Alu.is_equal)

            def fin_tile(n):
                t = pfin.tile([P, 1], f32, tag="fin")
                return t[0:n, :]

            pl128 = fin_tile(P)
            nc.tensor.matmul(pl128[:], b8T[:], len_f[:], start=True, stop=True)
            len128 = small.tile([P, 1], f32, tag="len128")
            nc.scalar.copy(len128[:], pl128[:])
            len128m1 = small.tile([P, 1], f32, tag="len128m1")
            nc.vector.tensor_scalar(len128m1[:], len128[:], -1.0, None,
                                    op0=Alu.add)

            # tv128[p, f] = t = (p & 7) * 64 + f
            i64 = small.tile([P, FS], i32, tag="i64")
            nc.gpsimd.iota(i64[:], pattern=[[1, FS]], base=0,
                           channel_multiplier=FS)
            piota = small.tile([P, 1], i32, tag="piota")
            nc.gpsimd.iota(piota[:], pattern=[[0, 1]], base=0,
                           channel_multiplier=1)
            bq = small.tile([P, 1], i32, tag="bq")
            nc.vector.tensor_scalar(bq[:], piota[:], 3, None,
                                    op0=Alu.arith_shift_right)
            boff = small.tile([P, 1], i32, tag="boff")
            nc.vector.tensor_scalar(boff[:], bq[:], 9, None,
                                    op0=Alu.logical_shift_left)
            bofff = small.tile([P, 1], f32, tag="bofff")
            nc.vector.tensor_copy(bofff[:], boff[:])
            tv128 = small.tile([P, FS], f32, tag="tv128")
            nc.vector.tensor_copy(tv128[:], i64[:])
            nc.vector.tensor_scalar(tv128[:], tv128[:], bofff[:], None,
                                    op0=Alu.subtract)

            mask128 = small.tile([P, FS], f32, tag="mask128")
            nc.vector.tensor_scalar(mask128[:], tv128[:], len128[:], None,
                                    op0=Alu.is_lt)
            omega = small.tile([P, FS], f32, tag="omega")
            nc.vector.tensor_scalar(omega[:], tv128[:], len128[:], None,
                                    op0=Alu.is_ge)

            idn_i = small.tile([P, P], i32, tag="idn_i")
            nc.gpsimd.iota(idn_i[:], pattern=[[1, P]], base=0,
                           channel_multiplier=-1)
            idn128 = small.tile([P, P], f32, tag="idn128")
            nc.vector.tensor_scalar(idn128[:], idn_i[:], 0.0, None,
                                    op0=Alu.is_equal)

            # ======== E slab, chunk-pipelined ========
            E = big.tile([P, SL * SL], f32, tag="E")
            E3 = E[:].rearrange("p (s j) -> p s j", j=SL)
            em3 = em_raw[:].rearrange("p (s j) -> p s j", j=J)
            s_sum = small.tile([P, FS], f32, tag="s_sum")
            s_eff = small.tile([P, FS], f32, tag="s_eff")
            rs = small.tile([P, FS], f32, tag="rs")
            c_log = small.tile([P, FS], f32, tag="c_log")
            csum4 = small.tile([P, 4], f32, tag="csum4")
            escs = big.tile([P, NQ * P], f32, tag="escs")

            def emit_exp(c):
                sl_c = slice(SC * c, SC * (c + 1))
                nc.scalar.activation(E3[:, sl_c, :J], em3[:, sl_c, :], Exp)

            def emit_ln(c):
                sl_c = slice(SC * c, SC * (c + 1))
                nc.scalar.activation(c_log[:, sl_c], s_eff[:, sl_c], Ln,
                                     accum_out=csum4[:, c : c + 1])

            def stats_ops(c, norm_eng=None):
                sl_c = slice(SC * c, SC * (c + 1))
                if norm_eng is None:
                    norm_eng = nc.gpsimd
                return [
                    lambda: nc.vector.tensor_reduce(
                        s_sum[:, sl_c], E3[:, sl_c, :J], axis=Ax.X,
                        op=Alu.add),
                    lambda: nc.vector.tensor_scalar(
                        s_eff[:, sl_c], s_sum[:, sl_c], -1.0, None,
                        op0=Alu.add),
                    lambda: nc.vector.tensor_mul(
                        s_eff[:, sl_c], s_eff[:, sl_c], mask128[:, sl_c]),
                    lambda: nc.vector.tensor_scalar(
                        s_eff[:, sl_c], s_eff[:, sl_c], 1.0, None,
                        op0=Alu.add),
                    lambda: nc.vector.reciprocal(rs[:, sl_c], s_eff[:, sl_c]),
                    lambda: nc.vector.tensor_mul(
                        rs[:, sl_c], rs[:, sl_c], mask128[:, sl_c]),
                    lambda: norm_eng.tensor_mul(
                        E3[:, sl_c, :J], E3[:, sl_c, :J],
                        rs[:, sl_c].unsqueeze(2).broadcast_to([P, SC, J])),
                    lambda: nc.vector.tensor_copy(
                        E3[:, sl_c, J:JA], omega[:, sl_c].unsqueeze(2)),
                ]

            # packed transposes: 4 q's share one PSUM bank, one copy per pack
            def emit_tpack(q4):
                # q4: four consecutive ascending q's sharing one PSUM bank
                pt = pesc.tile([P, 4 * P], f32, tag="pt")
                for i, q in enumerate(q4):
                    nc.tensor.matmul(pt[:, i * P : (i + 1) * P],
                                     E[:, P * q : P * q + P], idn128[:],
                                     is_transpose=True, start=True, stop=True)
                q0 = q4[0]
                nc.scalar.copy(escs[:, q0 * P : (q0 + 4) * P], pt[:])

            # chunks c0, c3, c2 fully before the chain; c1 in-loop
            emit_exp(0)
            emit_exp(3)
            emit_exp(2)
            emit_exp(1)
            for fn in stats_ops(0, norm_eng=nc.vector):
                fn()
            for fn in stats_ops(3):
                fn()
            emit_tpack([0, 1, 2, 3])
            emit_tpack([28, 29, 30, 31])
            emit_tpack([4, 5, 6, 7])
            emit_tpack([24, 25, 26, 27])
            for fn in stats_ops(2):
                fn()

            # ---- weights (on-chip from raw loads) ----
            W = small.tile([JA, JA], f32, tag="W")
            nc.vector.memset(W[:], 0.0)
            nc.scalar.activation(W[:J, :J], Traw[:], Exp)
            nc.scalar.activation(W[:J, J : J + 1], enC[:], Exp)
            nc.vector.memset(W[J : J + 1, J : J + 1], 1.0)

            ptt = pesc.tile([P, 4 * P], f32, tag="pt")
            nc.tensor.matmul(ptt[0:J, 0:J], Traw[:], idn128[0:J, 0:J],
                             is_transpose=True, start=True, stop=True)
            nc.tensor.matmul(ptt[0:1, P : P + J], enC[:], idn128[0:J, 0:J],
                             is_transpose=True, start=True, stop=True)
            Wt = small.tile([JA, JA], f32, tag="Wt")
            nc.vector.memset(Wt[:], 0.0)
            nc.scalar.activation(Wt[:J, :J], ptt[0:J, 0:J], Exp)
            nc.scalar.activation(Wt[J : J + 1, :J], ptt[0:1, P : P + J], Exp)
            nc.vector.memset(Wt[J : J + 1, J : J + 1], 1.0)

            est_aug = small.tile([JA, 1], f32, tag="est_aug")
            nc.vector.memset(est_aug[:], 0.0)
            nc.scalar.activation(est_aug[:J, :], st_raw[:], Exp)
            enp = small.tile([JA, 1], f32, tag="enp")
            nc.vector.memset(enp[:], 1.0)
            nc.scalar.activation(enp[:J, :], enC[:], Exp)

            emit_tpack([20, 21, 22, 23])
            emit_tpack([16, 17, 18, 19])
            # all Ln ops after every head Exp: at most one ACT table switch,
            # and it lands off the chain-critical path
            emit_ln(0)
            emit_ln(3)
            emit_ln(2)

            ones_ja = small.tile([JA, 1], f32, tag="ones_ja")
            nc.vector.memset(ones_ja[:], 1.0)
            ones_b = small.tile([B, 1], f32, tag="ones_b")
            nc.vector.memset(ones_b[:], 1.0)
            ones_j = small.tile([J, 1], f32, tag="ones_j")
            nc.vector.memset(ones_j[:], 1.0)
            ones_p = small.tile([P, 1], f32, tag="ones_p")
            nc.vector.memset(ones_p[:], 1.0)

            s_all = small.tile([1, 1], f32, tag="s_all")

            def emit_s_all():
                nc.gpsimd.tensor_reduce(s_all[:], csum4[:], axis=Ax.XYZWC,
                                        op=Alu.add)

            def e_ap(t):
                sl_, sh_ = t & 63, t >> 6
                q, r = sl_ >> 1, sl_ & 1
                v = escs[:, q * P : (q + 1) * P].rearrange(
                    "p (b s) -> p s b", b=B)
                return v[64 * r : 64 * r + JA, sh_, :]

            # ======== score: allocs + deferred emission helpers ========
            score_smalls = []
            if do_score:
                maskn128 = small.tile([P, FS], f32, tag="maskn128")
                last128 = small.tile([P, FS], f32, tag="last128")
                tgt128f = small.tile([P, FS], f32, tag="tgt128f")
                tgtn128f = small.tile([P, FS], f32, tag="tgtn128f")
                tgtmP = small.tile([P, FS], f32, tag="tgtmP")
                tgtmN = small.tile([P, FS], f32, tag="tgtmN")
                iota_ji = small.tile([P, J], i32, tag="iota_ji")
                nc.gpsimd.iota(iota_ji[:], pattern=[[1, J]], base=0,
                               channel_multiplier=0)
                iota_jf = small.tile([P, J], f32, tag="iota_jf")
                tgt0f = small.tile([B, 1], f32, tag="tgt0f")
                iota_jb = small.tile([B, J], i32, tag="iota_jb")
                nc.gpsimd.iota(iota_jb[:], pattern=[[1, J]], base=0,
                               channel_multiplier=0)
                iota_jbf = small.tile([B, J], f32, tag="iota_jbf")
                oh0 = small.tile([B, J], f32, tag="oh0")
                ohp = big.tile([P, FS * J], f32, tag="ohp")
                ohp3 = ohp[:].rearrange("p (f j) -> p f j", j=J)
                ohn = big.tile([P, FS * J], f32, tag="ohn")
                ohn3 = ohn[:].rearrange("p (f j) -> p f j", j=J)
                wsel = big.tile([P, FS * J], f32, tag="wsel")
                prod = big.tile([P, FS * J], f32, tag="prod")
                endtot_s = small.tile([1, 1], f32, tag="endtot_s")
                emtot_s = small.tile([1, 1], f32, tag="emtot_s")
                cpsum = pcp.tile([J, J], f32, tag="cpsum")

                score_smalls = [
                    lambda: nc.vector.tensor_scalar(
                        maskn128[:], tv128[:], len128m1[:], None,
                        op0=Alu.is_lt),
                    lambda: nc.vector.tensor_scalar(
                        last128[:], tv128[:], len128m1[:], None,
                        op0=Alu.is_equal),
                    lambda: nc.vector.tensor_copy(tgt128f[:], tgt128[:]),
                    lambda: nc.vector.tensor_copy(tgtn128f[:], tgtn128[:]),
                    lambda: nc.vector.tensor_copy(iota_jf[:], iota_ji[:]),
                    lambda: nc.vector.tensor_scalar(
                        tgtmP[:], tgt128f[:], 1.0, None, op0=Alu.add),
                    lambda: nc.vector.tensor_mul(tgtmP[:], tgtmP[:],
                                                 mask128[:]),
                    lambda: nc.vector.tensor_scalar(
                        tgtmP[:], tgtmP[:], -1.0, None, op0=Alu.add),
                    lambda: nc.vector.tensor_scalar(
                        tgtmN[:], tgtn128f[:], 1.0, None, op0=Alu.add),
                    lambda: nc.vector.tensor_mul(tgtmN[:], tgtmN[:],
                                                 maskn128[:]),
                    lambda: nc.vector.tensor_scalar(
                        tgtmN[:], tgtmN[:], -1.0, None, op0=Alu.add),
                    lambda: nc.vector.tensor_copy(tgt0f[:], tgt0[:]),
                    lambda: nc.vector.tensor_copy(iota_jbf[:], iota_jb[:]),
                    lambda: nc.vector.tensor_scalar(
                        oh0[:], iota_jbf[:], tgt0f[:], None,
                        op0=Alu.is_equal),
                ]

                def emit_oh_chunk(i):
                    # chunk i: 4 f's of ohp (even i) / ohn (odd i)
                    f0 = 4 * (i // 2)
                    dst3, code = (ohp3, tgtmP) if i % 2 == 0 else (ohn3, tgtmN)
                    nc.vector.tensor_tensor(
                        dst3[:, f0 : f0 + 4, :],
                        iota_jf[:].unsqueeze(1).broadcast_to([P, 4, J]),
                        code[:, f0 : f0 + 4].unsqueeze(2)
                        .broadcast_to([P, 4, J]),
                        op=Alu.is_equal,
                    )

                def emit_score_pool():
                    nc.gpsimd.tensor_tensor(
                        wsel[:].rearrange("p (f j) -> p f j", j=J), ohp3,
                        last128[:].unsqueeze(2).broadcast_to([P, FS, J]),
                        op=Alu.mult,
                    )
                    nc.gpsimd.tensor_tensor(
                        wsel[:].rearrange("p (f j) -> p f j", j=J),
                        wsel[:].rearrange("p (f j) -> p f j", j=J),
                        endJ[:].unsqueeze(1).broadcast_to([P, FS, J]),
                        op=Alu.mult,
                    )
                    nc.gpsimd.tensor_reduce(endtot_s[:], wsel[:],
                                            axis=Ax.XYZWC, op=Alu.add)
                    nc.gpsimd.tensor_mul(prod[:], ohp[:], em_raw[:])
                    nc.gpsimd.tensor_reduce(emtot_s[:], prod[:],
                                            axis=Ax.XYZWC, op=Alu.add)

            # ======== the bidirectional chain ========
            if do_scan:
                aF = apoolF.tile([JA, B], f32, tag="aF")
                nc.vector.tensor_scalar(aF[:], e_ap(0), est_aug[:], None,
                                        op0=Alu.mult)
                xB = apoolB.tile([JA, B], f32, tag="xB")
                nc.vector.tensor_scalar(xB[:], e_ap(2 * HF - 1), enp[:], None,
                                        op0=Alu.mult)
                psB = pscanB.tile([JA, B], f32, tag="psB")
                nc.tensor.matmul(psB[:], Wt[:], xB[:], start=True, stop=True)

                c1_stats = stats_ops(1)
                oh_sched = {}
                cp_sched = {}
                pool_at = -1
                if do_score:
                    for i in range(32):
                        oh_sched[28 + 2 * i] = i
                    pool_at = 94
                    if CPSUM_INLINE:
                        for f in range(FS):
                            cp_sched[98 + 2 * f] = f

                for k in range(1, HF):
                    psF = pscanF.tile([JA, B], f32, tag="psF")
                    nc.tensor.matmul(psF[:], W[:], aF[:], start=True,
                                     stop=True)
                    xB2 = apoolB.tile([JA, B], f32, tag="xB")
                    nc.vector.tensor_mul(xB2[:], psB[:], e_ap(2 * HF - 1 - k))
                    aF2 = apoolF.tile([JA, B], f32, tag="aF")
                    nc.vector.tensor_mul(aF2[:], psF[:], e_ap(k))
                    psB = pscanB.tile([JA, B], f32, tag="psB")
                    nc.tensor.matmul(psB[:], Wt[:], xB2[:], start=True,
                                     stop=True)
                    aF, xB = aF2, xB2
                    if k <= 8:
                        c1_stats[k - 1]()
                    elif k == 9:
                        emit_ln(1)
                    elif k == 10:
                        emit_tpack([8, 9, 10, 11])
                    elif k == 11:
                        emit_tpack([12, 13, 14, 15])
                    elif k == 12:
                        emit_s_all()
                    elif 13 <= k and k - 13 < len(score_smalls):
                        score_smalls[k - 13]()
                    if k in oh_sched:
                        emit_oh_chunk(oh_sched[k])
                    if k == pool_at:
                        emit_score_pool()
                    if k in cp_sched:
                        f = cp_sched[k]
                        nc.tensor.matmul(cpsum[:], ohp3[:, f, :],
                                         ohn3[:, f, :], start=(f == 0),
                                         stop=(f == FS - 1))

                if do_score and not CPSUM_INLINE:
                    for f in range(FS):
                        nc.tensor.matmul(cpsum[:], ohp3[:, f, :],
                                         ohn3[:, f, :], start=(f == 0),
                                         stop=(f == FS - 1))

                # ---- meet: Z_b = v_255^T A_255 ----
                rm = apoolF.tile([JA, B], f32, tag="rm")
                nc.vector.tensor_mul(rm[:], psB[:], aF[:])
                zrow_t = pscanF.tile([JA, B], f32, tag="psF")
                zrow = zrow_t[0:1, :]
                nc.tensor.matmul(zrow, ones_ja[:], rm[:], start=True,
                                 stop=True)
                logz = small.tile([1, B], f32, tag="logz")
                nc.scalar.activation(logz[:], zrow, Ln)
            else:
                for fn in stats_ops(1):
                    fn()
                emit_ln(1)
                emit_tpack([8, 9, 10, 11])
                emit_tpack([12, 13, 14, 15])
                emit_s_all()
                for fn in score_smalls:
                    fn()
                if do_score:
                    for i in range(32):
                        emit_oh_chunk(i)
                    emit_score_pool()
                    for f in range(FS):
                        nc.tensor.matmul(cpsum[:], ohp3[:, f, :],
                                         ohn3[:, f, :], start=(f == 0),
                                         stop=(f == FS - 1))

            # ======== combine ========
            res = small.tile([1, 1], f32, tag="res")
            if do_scan:
                s2 = small.tile([1, 1], f32, tag="s2")
                nc.vector.tensor_reduce(s2[:], logz[:], axis=Ax.X, op=Alu.add)
                nc.vector.tensor_add(res[:], s_all[:], s2[:])
            else:
                nc.vector.tensor_copy(res[:], s_all[:])

            if do_score:
                # pfin has bufs=1: each tile's reads precede the next tile call
                cnt0 = fin_tile(J)
                nc.tensor.matmul(cnt0[:], oh0[:], ones_b[:], start=True,
                                 stop=True)
                stsc = small.tile([J, 1], f32, tag="stsc")
                nc.vector.tensor_mul(stsc[:], cnt0[:], st_raw[:])
                nc.vector.tensor_sub(res[:], res[:], emtot_s[:])
                nc.vector.tensor_sub(res[:], res[:], endtot_s[:])
                tscratch = small.tile([J, J], f32, tag="tscratch")
                nc.vector.tensor_mul(tscratch[:], cpsum[:], Traw[:])
                tsc = small.tile([J, 1], f32, tag="tsc")
                nc.vector.tensor_reduce(tsc[:], tscratch[:], axis=Ax.X,
                                        op=Alu.add)
                sneg = small.tile([J, 1], f32, tag="sneg")
                nc.vector.tensor_add(sneg[:], tsc[:], stsc[:])
                nc.vector.tensor_scalar(sneg[:], sneg[:], -1.0, None,
                                        op0=Alu.mult)
                s3p = fin_tile(1)
                nc.tensor.matmul(s3p[:], ones_j[:], sneg[:], start=True,
                                 stop=True)
                nc.vector.tensor_add(res[:], res[:], s3p[:])
            nc.sync.dma_start(out_d.ap(), res[:])

    nc.compile()
    return nc


_NC_CACHE = None


def kernel(emission, length, target, transition, start_transition,
           end_transition):
    global _NC_CACHE
    from concourse.bass_utils import run_bass_kernel_spmd

    emission = np.ascontiguousarray(np.asarray(emission, np.float32))
    length = np.asarray(length).astype(np.int32).reshape(-1, 1)
    target = np.asarray(target).astype(np.int32)
    transition = np.ascontiguousarray(np.asarray(transition, np.float32))
    start = np.asarray(start_transition, np.float32).reshape(J, 1)
    end = np.asarray(end_transition, np.float32).reshape(J, 1)

    if _NC_CACHE is None:
        _NC_CACHE = build_bass()
    nc = _NC_CACHE

    in_maps = []
    for c in range(NCORES):
        sl = slice(c * B, (c + 1) * B)
        in_maps.append({
            "emission": np.ascontiguousarray(emission[sl]),
            "length": np.ascontiguousarray(length[sl]),
            "target": np.ascontiguousarray(target[sl]),
            "transition": transition,
            "start_transition": start,
            "end_transition": end,
        })

    r = run_bass_kernel_spmd(nc, in_maps, list(range(NCORES)))
    total = np.float64(0.0)
    for c in range(NCORES):
        total += np.float64(r.results[c]["out"][0, 0])
    return np.asarray(total, np.float32)


if __name__ == "__main__":
    rng = np.random.default_rng(0)
    inputs = {
        "emission": rng.standard_normal((128, S, J)).astype(np.float32),
        "length": rng.integers(2, S + 1, size=(128,)),
        "target": rng.integers(0, J, size=(128, S)),
        "transition": (rng.standard_normal((J, J)) * 0.1).astype(np.float32),
        "start_transition": (rng.standard_normal(J) * 0.1).astype(np.float32),
        "end_transition": (rng.standard_normal(J) * 0.1).astype(np.float32),
    }
    print(kernel(**inputs))
